# revision 1
# baseline (speedup 1.0000x reference)
"""Trainium2 Bass kernel for nn_A3TGCN2_EdgeClassifier (GNN message passing).

Math (validated vs reference in fp32): with H0 = 0 the GRU collapses
(R drops out; softmax over one period == 1):
    deg[d] = 1 + sum_{e: dst=d} ew[e];   dinv = deg^-1/2
    Y  = X @ [Wz@lzW[:64] | Wh@lhW[:64]]              (N,128)
    Ys = dinv * Y
    Yagg[d] = dinv[d] * ( sum_e ew[e]*Ys[src[e]] + Ys[d] )
    Z = sigmoid(Yagg[:,:64] + bz');  Ht = tanh(Yagg[:,64:] + bh')
    h = (1-Z)*Ht;  A = h@W1[:64];  B = h@W1[64:] + b1
    out[e] = relu(A[esrc]+B[edst]) @ W2 + b2          (E,2)

Distribution: nodes sharded across 8 cores (12544 each); each core builds
its slice of the (N,128) node tables, all-gathered via collectives. Per-edge
row gathers use the gpsimd dma_gather ucode op (InstDMAGatherAnt): up to
1024 rows of 256B per instruction at ~994ns + 0.34ns/row — ~6.5x cheaper in
descriptor-generation time than per-128-row indirect DMA. dma_gather takes
int16 indices, so edge chunks are bucketed by table quarter (25088 rows)
and gathered from a quarter-offset source AP. The GCN aggregation is
sharded by dst ownership: chunks of 128 dst-sorted edges scatter via
one-hot matmul into per-dst-tile PSUM accumulators (grouped GT tiles per
PSUM residency group, quarter-major chunk order inside a group so gather
calls stay within one quarter). The edge MLP is sharded by edge-dst owner:
B[edst] is expanded from the local table by one-hot matmul; A[esrc] rows
come from batched dma_gather of full AB rows (cols 0:64 = A).
"""

import sys

try:
    import concourse.bass as bass  # noqa: F401
except Exception:  # pragma: no cover
    sys.path.insert(0, "/opt/trn_rl_repo")

import numpy as np
import ml_dtypes

import concourse.bass as bass
import concourse.mybir as mybir
from concourse import bacc, tile
from concourse.bass_utils import run_bass_kernel_spmd

BF16 = ml_dtypes.bfloat16
F32 = np.float32

NCORES = 8
N = 100_000
E = 1_600_000
FIN = 80
NLOC = 12544               # padded nodes per core
NPAD = NLOC * NCORES       # 100352
TPC = NLOC // 128          # 98 node tiles per core
GT = 6                     # node tiles per aggregation group (PSUM residency)
QS = NPAD // 4             # table quarter for int16 gather indices (25088)
NG = 8                     # max chunks (x128 rows) per dma_gather call
EPC = E // NCORES          # 200000

dt = mybir.dt


def _mk_calls(stream_q):
    """Split a chunk stream [(chunk_meta, key), ...] into dma_gather calls of
    <=NG chunks with a constant split key (whose last element is the table
    quarter q). Returns list of (q, [chunk_meta,...])."""
    calls = []
    cur_k, cur = None, []
    for meta, key in stream_q:
        if key != cur_k or len(cur) == NG:
            if cur:
                calls.append((cur_k[-1], cur))
            cur_k, cur = key, []
        cur.append(meta)
    if cur:
        calls.append((cur_k[-1], cur))
    return calls


def _slots(t_k, q_k, colof):
    """Vectorized slot assignment: edges -> (col, par) within (t, q) chunks."""
    n = len(t_k)
    key = t_k * 4 + q_k
    order2 = np.lexsort((np.arange(n), key))
    cnt1d = np.bincount(key, minlength=TPC * 4)
    bstart = np.zeros(TPC * 4, np.int64)
    bstart[1:] = np.cumsum(cnt1d)[:-1]
    skey = key[order2]
    pos = np.arange(n) - bstart[skey]
    col = np.zeros(n, np.int64)
    par = np.zeros(n, np.int64)
    col[order2] = colof.reshape(-1)[skey] + pos // 128
    par[order2] = pos % 128
    return col, par


def _prep(inputs):
    """Host-side sharding / graph preprocessing. Returns (in_maps, meta)."""
    x = np.asarray(inputs["x"], F32)[0, :, :, 0]
    ei = np.asarray(inputs["edge_index"]).astype(np.int64)
    src, dst = ei[0], ei[1]
    ew = np.asarray(inputs["edge_weight"], F32)
    esrc = np.asarray(inputs["edge_src"]).astype(np.int64)
    edst = np.asarray(inputs["edge_dst"]).astype(np.int64)

    lzW = np.asarray(inputs["lz_W"], F32)[:64]
    lhW = np.asarray(inputs["lh_W"], F32)[:64]
    Wzp = np.asarray(inputs["Wz"], F32) @ lzW
    Whp = np.asarray(inputs["Wh"], F32) @ lhW
    bzp = np.asarray(inputs["bz"], F32) @ lzW + np.asarray(inputs["lz_b"], F32)
    bhp = np.asarray(inputs["bh"], F32) @ lhW + np.asarray(inputs["lh_b"], F32)
    W1 = np.asarray(inputs["mlp_W1"], F32)
    b1 = np.asarray(inputs["mlp_b1"], F32)
    W2 = np.asarray(inputs["mlp_W2"], F32)
    b2 = np.asarray(inputs["mlp_b2"], F32)

    wfused = np.ascontiguousarray(np.concatenate([Wzp, Whp], 1)).astype(BF16)
    gbias = np.tile(np.concatenate([bzp, bhp])[None, :], (128, 1)).astype(F32)
    w1cat = np.ascontiguousarray(np.concatenate([W1[:64], W1[64:]], 1)).astype(BF16)
    b1row = np.concatenate([np.zeros(64, F32), b1])[None, :].astype(BF16)
    w2sb = W2.astype(BF16)
    b2rep = np.tile(b2[None, :], (128, 64)).astype(F32)
    iota = np.tile(np.arange(128, dtype=F32)[None, :], (128, 1)).astype(BF16)
    ident = np.eye(128, dtype=F32).astype(BF16)
    ones1 = np.ones((1, 128), BF16)
    iotap = np.arange(128, dtype=F32).reshape(128, 1)

    xpad = np.zeros((NPAD, FIN), F32)
    xpad[:N] = x
    xts = [np.ascontiguousarray(xpad[k * NLOC:(k + 1) * NLOC].T).astype(BF16)
           for k in range(NCORES)]

    counts = np.bincount(dst, minlength=NPAD)
    L = int(counts.max())
    order = np.argsort(dst, kind="stable")
    dsts = dst[order]
    ews = ew[order]
    srcs = src[order]
    cum = np.zeros(NPAD + 1, np.int64)
    cum[1:] = np.cumsum(counts)
    ewpad = np.zeros((NPAD, L), F32)
    posn = np.arange(E) - cum[dsts]
    ewpad[dsts, posn] = ews
    ewpads = [np.ascontiguousarray(
        ewpad[k * NLOC:(k + 1) * NLOC]
        .reshape(TPC, 128, L).transpose(1, 0, 2).reshape(128, TPC * L))
        for k in range(NCORES)]

    # ---- aggregation streams: chunk per (dst tile, src quarter) ----
    k_of = dsts // NLOC
    t_of = (dsts % NLOC) // 128
    q_of = srcs // QS
    cnt = np.zeros((NCORES, TPC, 4), np.int64)
    np.add.at(cnt, (k_of, t_of, q_of), 1)
    capsq = (cnt.max(0) + 127) // 128              # [TPC, 4]
    for t in range(TPC):
        if capsq[t].sum() == 0:
            capsq[t][0] = 1

    ngroups = (TPC + GT - 1) // GT
    groups = [list(range(g * GT, min((g + 1) * GT, TPC))) for g in range(ngroups)]

    # chunk stream order: group g -> quarter q -> tile t in g -> chunk cc
    # chunk meta: (c, t, q) with start/stop flags computed from per-tile order
    stream = []                 # (c, t, q)
    c = 0
    for grp in groups:
        for q in range(4):
            for t in grp:
                for _ in range(int(capsq[t][q])):
                    stream.append((c, t, q))
                    c += 1
    CTOT = c
    colof = np.full((TPC, 4), -1, np.int64)
    for (cc, t, q) in stream:
        if colof[t][q] < 0:
            colof[t][q] = cc

    # start/stop flags per chunk (psum accumulate lifetime = per tile)
    first_of_t, last_of_t = {}, {}
    for (cc, t, q) in stream:
        if t not in first_of_t:
            first_of_t[t] = cc
        last_of_t[t] = cc
    chunk_flags = [(cc, t, q, cc == first_of_t[t], cc == last_of_t[t])
                   for (cc, t, q) in stream]

    # gather calls (same for all cores); split on (psum group, quarter)
    gidx_of_tile = {}
    for gi, grp in enumerate(groups):
        for t in grp:
            gidx_of_tile[t] = gi
    calls_agg = _mk_calls(
        [((cc, t, cc == first_of_t[t], cc == last_of_t[t]), (gidx_of_tile[t], q))
         for (cc, t, q) in stream])
    AIW = sum(len(ch) * 8 for _, ch in calls_agg)   # int16 cols in idx stream

    # per-core stream data
    agg = []
    for k in range(NCORES):
        sel = slice(cum[k * NLOC], cum[(k + 1) * NLOC])
        d_k = dsts[sel]
        s_k = srcs[sel]
        e_k = ews[sel]
        t_k = (d_k - k * NLOC) // 128
        q_k = s_k // QS
        slot_col, slot_par = _slots(t_k, q_k, colof)
        dstrel = np.zeros((128, CTOT), F32)
        ewagg = np.zeros((128, CTOT), F32)
        srcg = np.zeros((128, CTOT), np.int64)   # global src per slot (pad=q*QS)
        # init pads to the chunk's quarter base so rel idx is 0
        for (cc, t, q) in stream:
            srcg[:, cc] = q * QS
        dstrel[slot_par, slot_col] = ((d_k - k * NLOC) % 128).astype(F32)
        ewagg[slot_par, slot_col] = e_k
        srcg[slot_par, slot_col] = s_k
        # idx16 stream in call order, wrapped [16, w*8] replicated to 128 rows
        blocks = []
        for q, chs in calls_agg:
            cols = [m[0] for m in chs]
            vals = (srcg[:, cols].T.reshape(-1) - q * QS).astype(np.int16)
            w16 = vals.reshape(-1, 16).T            # [16, w*8]
            blocks.append(np.tile(w16, (8, 1)))
        aggidx = np.concatenate(blocks, axis=1)
        assert aggidx.shape == (128, AIW)
        agg.append((dstrel, ewagg, aggidx))

    # ---- MLP streams: chunk per (edst tile, esrc quarter), B expanded ----
    morder = np.argsort(edst, kind="stable")
    medst = edst[morder]
    mesrc = esrc[morder]
    mcum = np.zeros(NPAD + 1, np.int64)
    mcum[1:] = np.cumsum(np.bincount(medst, minlength=NPAD))
    mk_of = medst // NLOC
    mt_of = (medst % NLOC) // 128
    mq_of = mesrc // QS
    cntm = np.zeros((NCORES, TPC, 4), np.int64)
    np.add.at(cntm, (mk_of, mt_of, mq_of), 1)
    capm = (cntm.max(0) + 127) // 128               # [TPC, 4], 0 allowed

    # chunk order: quarter-major, then tile
    mstream = []
    mcolof = np.full((TPC, 4), -1, np.int64)
    c = 0
    for q in range(4):
        for t in range(TPC):
            for _ in range(int(capm[t][q])):
                if mcolof[t][q] < 0:
                    mcolof[t][q] = c
                mstream.append((c, t, q))
                c += 1
    CTOTM = c
    calls_mlp = _mk_calls([((cc, t), (q,)) for (cc, t, q) in mstream])
    MIW = sum(len(ch) * 8 for _, ch in calls_mlp)

    mlp = []
    for k in range(NCORES):
        sel = slice(mcum[k * NLOC], mcum[(k + 1) * NLOC])
        d_k = medst[sel]
        s_k = mesrc[sel]
        t_k = (d_k - k * NLOC) // 128
        q_k = s_k // QS
        slot_col, slot_par = _slots(t_k, q_k, mcolof)
        drm = np.zeros((128, CTOTM), np.float32)
        srcg = np.zeros((128, CTOTM), np.int64)
        for (cc, t, q) in mstream:
            srcg[:, cc] = q * QS
        perm = np.full(CTOTM * 128, -1, np.int64)
        drm[slot_par, slot_col] = ((d_k - k * NLOC) % 128).astype(np.float32)
        srcg[slot_par, slot_col] = s_k
        perm[slot_col * 128 + slot_par] = morder[sel]
        blocks = []
        for q, chs in calls_mlp:
            cols = [m[0] for m in chs]
            vals = (srcg[:, cols].T.reshape(-1) - q * QS).astype(np.int16)
            w16 = vals.reshape(-1, 16).T
            blocks.append(np.tile(w16, (8, 1)))
        mlpidx = np.concatenate(blocks, axis=1)
        assert mlpidx.shape == (128, MIW)
        mlp.append((drm.astype(BF16), mlpidx, perm))

    NPIECE = (CTOTM + 63) // 64

    consts = dict(wfused=wfused, gbias=gbias, w1cat=w1cat, b1row=b1row,
                  w2sb=w2sb, b2rep=b2rep, iota=iota, iotap=iotap, ident=ident,
                  ones1=ones1)
    in_maps = []
    for k in range(NCORES):
        dstrel, ewagg, aggidx = agg[k]
        drow, mlpidx, _ = mlp[k]
        in_maps.append(dict(
            xt=xts[k], ewpad=ewpads[k], dstrel=dstrel, ewagg=ewagg,
            aggidx=aggidx, mdrow=drow, mlpidx=mlpidx, **consts))

    meta = dict(L=L, groups=groups, CTOT=CTOT, CTOTM=CTOTM,
                calls_agg=calls_agg, calls_mlp=calls_mlp, AIW=AIW, MIW=MIW,
                NPIECE=NPIECE, perms=[m[2] for m in mlp])
    return in_maps, meta


def _build(meta):
    L = meta["L"]
    groups = meta["groups"]
    CTOT = meta["CTOT"]
    CTOTM = meta["CTOTM"]
    calls_agg = meta["calls_agg"]
    calls_mlp = meta["calls_mlp"]
    AIW = meta["AIW"]
    MIW = meta["MIW"]
    NPIECE = meta["NPIECE"]

    nc = bacc.Bacc("TRN2", target_bir_lowering=False, debug=False,
                   num_devices=NCORES, num_swdge_queues=4)

    xt_d = nc.dram_tensor("xt", [FIN, NLOC], dt.bfloat16, kind="ExternalInput")
    ewpad_d = nc.dram_tensor("ewpad", [128, TPC * L], dt.float32, kind="ExternalInput")
    dstrel_d = nc.dram_tensor("dstrel", [128, CTOT], dt.float32, kind="ExternalInput")
    ewagg_d = nc.dram_tensor("ewagg", [128, CTOT], dt.float32, kind="ExternalInput")
    aggidx_d = nc.dram_tensor("aggidx", [128, AIW], dt.int16, kind="ExternalInput")
    mdrow_d = nc.dram_tensor("mdrow", [128, CTOTM], dt.bfloat16, kind="ExternalInput")
    mlpidx_d = nc.dram_tensor("mlpidx", [128, MIW], dt.int16, kind="ExternalInput")
    iotap_d = nc.dram_tensor("iotap", [128, 1], dt.float32, kind="ExternalInput")
    wfused_d = nc.dram_tensor("wfused", [FIN, 128], dt.bfloat16, kind="ExternalInput")
    gbias_d = nc.dram_tensor("gbias", [128, 128], dt.float32, kind="ExternalInput")
    w1cat_d = nc.dram_tensor("w1cat", [64, 128], dt.bfloat16, kind="ExternalInput")
    b1row_d = nc.dram_tensor("b1row", [1, 128], dt.bfloat16, kind="ExternalInput")
    w2sb_d = nc.dram_tensor("w2sb", [64, 2], dt.bfloat16, kind="ExternalInput")
    b2rep_d = nc.dram_tensor("b2rep", [128, 128], dt.float32, kind="ExternalInput")
    iota_d = nc.dram_tensor("iota", [128, 128], dt.bfloat16, kind="ExternalInput")
    ident_d = nc.dram_tensor("ident", [128, 128], dt.bfloat16, kind="ExternalInput")
    ones1_d = nc.dram_tensor("ones1", [1, 128], dt.bfloat16, kind="ExternalInput")
    out_d = nc.dram_tensor("out", [NPIECE * 128, 128], dt.float32, kind="ExternalOutput")

    ysloc = nc.dram_tensor("ysloc", [NLOC, 128], dt.bfloat16)
    ysfull = nc.dram_tensor("ysfull", [NPAD, 128], dt.bfloat16, addr_space="Shared")
    abloc = nc.dram_tensor("abloc", [NLOC, 128], dt.bfloat16)
    abfull = nc.dram_tensor("abfull", [NPAD, 128], dt.bfloat16, addr_space="Shared")

    rg = [list(range(NCORES))]

    with tile.TileContext(nc) as tc:
        with (
            tc.tile_pool(name="const", bufs=1) as cp,
            tc.tile_pool(name="persist", bufs=1) as pp,
        ):
            wfused_sb = cp.tile([FIN, 128], dt.bfloat16)
            gbias_sb = cp.tile([128, 128], dt.float32)
            w1cat_sb = cp.tile([64, 128], dt.bfloat16)
            b1row_sb = cp.tile([1, 128], dt.bfloat16)
            w2_sb = cp.tile([64, 2], dt.bfloat16)
            iotap_sb = cp.tile([128, 1], dt.float32)
            b2rep_sb = cp.tile([128, 128], dt.float32)
            iota_sb = cp.tile([128, 128], dt.bfloat16)
            ident_sb = cp.tile([128, 128], dt.bfloat16)
            ones1_sb = cp.tile([1, 128], dt.bfloat16)
            for sb, d in [
                (wfused_sb, wfused_d), (gbias_sb, gbias_d), (w1cat_sb, w1cat_d),
                (b1row_sb, b1row_d), (w2_sb, w2sb_d), (b2rep_sb, b2rep_d),
                (iota_sb, iota_d), (ident_sb, ident_d), (ones1_sb, ones1_d),
                (iotap_sb, iotap_d),
            ]:
                nc.sync.dma_start(out=sb[:], in_=d[:])

            dinv = pp.tile([128, TPC], dt.float32)
            dinv2 = pp.tile([128, TPC], dt.float32)
            ys2b = pp.tile([128, TPC * 128], dt.bfloat16)

            # ---------------- deg / dinv ----------------
            with tc.tile_pool(name="degp", bufs=1) as dp:
                ewpad_sb = dp.tile([128, TPC * L], dt.float32)
                nc.sync.dma_start(out=ewpad_sb[:], in_=ewpad_d[:])
                deg = dp.tile([128, TPC], dt.float32)
                nc.vector.tensor_reduce(
                    deg[:], ewpad_sb[:].rearrange("p (t l) -> p t l", t=TPC),
                    axis=mybir.AxisListType.X, op=mybir.AluOpType.add)
                sq = dp.tile([128, TPC], dt.float32)
                nc.scalar.activation(sq[:], deg[:],
                                     mybir.ActivationFunctionType.Sqrt, bias=1.0)
                nc.vector.reciprocal(dinv[:], sq[:])
                nc.vector.tensor_mul(dinv2[:], dinv[:], dinv[:])

            # ---------------- node phase ----------------
            with (
                tc.tile_pool(name="xtp", bufs=1) as xtp,
                tc.tile_pool(name="ysp", bufs=1) as ysp,
                tc.tile_pool(name="pY", bufs=4, space="PSUM") as pYp,
            ):
                xt_all = xtp.tile([FIN, NLOC], dt.bfloat16)
                nc.sync.dma_start(out=xt_all[:], in_=xt_d[:])
                ys_all = ysp.tile([128, TPC * 128], dt.bfloat16)
                for t in range(TPC):
                    pY = pYp.tile([128, 128], dt.float32)
                    nc.tensor.matmul(pY[:], xt_all[:, t * 128:(t + 1) * 128],
                                     wfused_sb[:], start=True, stop=True)
                    nc.vector.tensor_scalar_mul(
                        ys_all[:, t * 128:(t + 1) * 128], pY[:], dinv[:, t:t + 1])
                    nc.vector.scalar_tensor_tensor(
                        ys2b[:, t * 128:(t + 1) * 128], pY[:], dinv2[:, t:t + 1],
                        gbias_sb[:], op0=mybir.AluOpType.mult, op1=mybir.AluOpType.add)
                nc.sync.dma_start(
                    out=ysloc[:].rearrange("(t p) f -> p t f", p=128),
                    in_=ys_all[:].rearrange("p (t f) -> p t f", t=TPC))

            nc.gpsimd.collective_compute(
                "AllGather", mybir.AluOpType.bypass, replica_groups=rg,
                ins=[ysloc[:]], outs=[ysfull[:]])

            # ---------------- aggregation + gates + A|B ----------------
            with (
                tc.tile_pool(name="aggstream", bufs=1) as asp,
                tc.tile_pool(name="gat", bufs=8) as gatp,
                tc.tile_pool(name="oh", bufs=8) as ohp,
                tc.tile_pool(name="gate", bufs=3) as gp_,
                tc.tile_pool(name="abp", bufs=3) as abp,
                tc.tile_pool(name="pAgg", bufs=GT, space="PSUM") as pAgg,
                tc.tile_pool(name="pT", bufs=1, space="PSUM") as pT,
                tc.tile_pool(name="pAB", bufs=1, space="PSUM") as pAB,
            ):
                dstrel_sb = asp.tile([128, CTOT], dt.float32)
                ewagg_sb = asp.tile([128, CTOT], dt.float32)
                aggidx_sb = asp.tile([128, AIW], dt.int16)
                nc.sync.dma_start(out=dstrel_sb[:], in_=dstrel_d[:])
                nc.sync.dma_start(out=ewagg_sb[:], in_=ewagg_d[:])
                nc.sync.dma_start(out=aggidx_sb[:], in_=aggidx_d[:])

                # group calls by psum group: tiles of group g span calls whose
                # chunks' tiles are all in g (stream order guarantees this)
                gidx_of_tile = {}
                for gi, grp in enumerate(groups):
                    for t in grp:
                        gidx_of_tile[t] = gi
                iw = 0
                cur_g = -1
                ps = {}
                for ci, (q, chs) in enumerate(calls_agg):
                    w = len(chs)
                    gi = gidx_of_tile[chs[0][1]]
                    if gi != cur_g:
                        # allocate psum tiles for the new group
                        cur_g = gi
                        ps = {t: pAgg.tile([128, 128], dt.float32, tag="aggpsum",
                                           name=f"ps_g{gi}_t{t}")
                              for t in groups[gi]}
                    gt_ = gatp.tile([128, NG * 128], dt.bfloat16, tag="gat")
                    nc.gpsimd.dma_gather(
                        out_ap=gt_[:, :w * 128].rearrange("p (b f) -> p b f", b=w),
                        in_ap=ysfull[q * QS:(q + 1) * QS, :],
                        idxs_ap=aggidx_sb[:, iw:iw + w * 8],
                        num_idxs=w * 128, num_idxs_reg=w * 128,
                        elem_size=128, transpose=False,
                        single_packet=False, queue_num=ci % 4)
                    iw += w * 8
                    for j, (cc, t, st, sp) in enumerate(chs):
                        oh = ohp.tile([128, 128], dt.bfloat16)
                        nc.vector.tensor_scalar(
                            oh[:], iota_sb[:], dstrel_sb[:, cc:cc + 1],
                            ewagg_sb[:, cc:cc + 1],
                            op0=mybir.AluOpType.is_equal,
                            op1=mybir.AluOpType.mult)
                        nc.tensor.matmul(ps[t][:], oh[:],
                                         gt_[:, j * 128:(j + 1) * 128],
                                         start=st, stop=sp)
                        if sp:
                            # epilogue for tile t
                            yagg = gp_.tile([128, 128], dt.float32, tag="yagg")
                            nc.vector.scalar_tensor_tensor(
                                yagg[:], ps[t][:], dinv[:, t:t + 1],
                                ys2b[:, t * 128:(t + 1) * 128],
                                op0=mybir.AluOpType.mult, op1=mybir.AluOpType.add)
                            zt = gp_.tile([128, 64], dt.float32, tag="zt")
                            ht = gp_.tile([128, 64], dt.float32, tag="ht")
                            nc.scalar.activation(zt[:], yagg[:, 0:64],
                                                 mybir.ActivationFunctionType.Sigmoid)
                            nc.scalar.activation(ht[:], yagg[:, 64:128],
                                                 mybir.ActivationFunctionType.Tanh)
                            zh = gp_.tile([128, 64], dt.float32, tag="zh")
                            nc.vector.tensor_mul(zh[:], zt[:], ht[:])
                            hbf = gp_.tile([128, 64], dt.bfloat16, tag="hbf")
                            nc.vector.tensor_sub(hbf[:], ht[:], zh[:])
                            psT = pT.tile([64, 128], dt.bfloat16)
                            nc.tensor.transpose(psT[:], hbf[:], ident_sb[:])
                            hT = gp_.tile([64, 128], dt.bfloat16, tag="hT")
                            nc.vector.tensor_copy(hT[:], psT[:])
                            psAB = pAB.tile([128, 128], dt.float32)
                            nc.tensor.matmul(psAB[:], ones1_sb[:], b1row_sb[:],
                                             start=True, stop=False)
                            nc.tensor.matmul(psAB[:], hT[:], w1cat_sb[:],
                                             start=False, stop=True)
                            ab = abp.tile([128, 128], dt.bfloat16)
                            nc.scalar.copy(ab[:], psAB[:])
                            nc.sync.dma_start(
                                out=abloc[t * 128:(t + 1) * 128, :], in_=ab[:])

            nc.gpsimd.collective_compute(
                "AllGather", mybir.AluOpType.bypass, replica_groups=rg,
                ins=[abloc[:]], outs=[abfull[:]])

            # ---------------- MLP phase (A gathered, B expanded) ----------------
            with (
                tc.tile_pool(name="mstream", bufs=1) as msp,
                tc.tile_pool(name="bwinp", bufs=1) as bwp,
                tc.tile_pool(name="ag", bufs=8) as agp,
                tc.tile_pool(name="ohn", bufs=6) as ohnp,
                tc.tile_pool(name="hT2", bufs=8) as hT2p,
                tc.tile_pool(name="op", bufs=3) as op_,
                tc.tile_pool(name="pO", bufs=2, space="PSUM") as pOp,
                tc.tile_pool(name="pBC", bufs=3, space="PSUM") as pBCp,
                tc.tile_pool(name="pE", bufs=3, space="PSUM") as pEp,
            ):
                bwin_all = bwp.tile([128, TPC * 64], dt.bfloat16)
                nc.sync.dma_start(
                    out=bwin_all[:].rearrange("p (t f) -> p t f", t=TPC),
                    in_=abloc[:, 64:128].rearrange("(t p) f -> p t f", p=128))
                mdrow_sb = msp.tile([128, CTOTM], dt.bfloat16)
                mlpidx_sb = msp.tile([128, MIW], dt.int16)
                nc.sync.dma_start(out=mdrow_sb[:], in_=mdrow_d[:])
                nc.sync.dma_start(out=mlpidx_sb[:], in_=mlpidx_d[:])
                MG = 64
                pO = None
                iw = 0
                for ci, (q, chs) in enumerate(calls_mlp):
                    w = len(chs)
                    ag = agp.tile([128, NG * 128], dt.bfloat16, tag="ag")
                    nc.gpsimd.dma_gather(
                        out_ap=ag[:, :w * 128].rearrange("p (b f) -> p b f", b=w),
                        in_ap=abfull[q * QS:(q + 1) * QS, :],
                        idxs_ap=mlpidx_sb[:, iw:iw + w * 8],
                        num_idxs=w * 128, num_idxs_reg=w * 128,
                        elem_size=128, transpose=False,
                        single_packet=False, queue_num=ci % 4)
                    iw += w * 8
                    for j, (cg, t) in enumerate(chs):
                        b = cg % MG
                        if b == 0:
                            pO = pOp.tile([128, 128], dt.float32)
                        psBC = pBCp.tile([128, 128], dt.bfloat16)
                        nc.tensor.transpose(
                            psBC[:],
                            mdrow_sb[:, cg:cg + 1].to_broadcast([128, 128]),
                            ident_sb[:])
                        ohn = ohnp.tile([128, 128], dt.bfloat16, tag="ohn")
                        nc.vector.tensor_scalar(
                            ohn[:], psBC[:], iotap_sb[:, 0:1], None,
                            op0=mybir.AluOpType.is_equal)
                        psE = pEp.tile([64, 128], dt.float32)
                        nc.tensor.matmul(psE[:], bwin_all[:, t * 64:(t + 1) * 64],
                                         ohn[:], start=True, stop=False)
                        nc.tensor.matmul(psE[:], ag[:, j * 128:j * 128 + 64],
                                         ident_sb[:], start=False, stop=True)
                        hTm = hT2p.tile([64, 128], dt.bfloat16)
                        nc.scalar.activation(hTm[:], psE[:],
                                             mybir.ActivationFunctionType.Relu)
                        nc.tensor.matmul(pO[:, 2 * b:2 * b + 2], hTm[:], w2_sb[:],
                                         start=True, stop=True)
                        if b == MG - 1 or cg == CTOTM - 1:
                            nb = b + 1
                            osb = op_.tile([128, 128], dt.float32)
                            nc.vector.tensor_add(osb[:, :2 * nb], pO[:, :2 * nb],
                                                 b2rep_sb[:, :2 * nb])
                            qq = cg // MG
                            nc.sync.dma_start(
                                out=out_d[qq * 128:(qq + 1) * 128, :], in_=osb[:])

    nc.compile()
    return nc


def _unshard(results, meta):
    CTOTM = meta["CTOTM"]
    out = np.zeros((E, 2), F32)
    for k in range(NCORES):
        outd = np.asarray(results[k]["out"])
        nslots = CTOTM * 128
        stream = np.empty((nslots, 2), F32)
        for q in range((CTOTM + 63) // 64):
            kk = min(64, CTOTM - q * 64)
            blk = outd[q * 128:(q + 1) * 128, :2 * kk]
            stream[q * 64 * 128:(q * 64 + kk) * 128] = (
                blk.reshape(128, kk, 2).transpose(1, 0, 2).reshape(kk * 128, 2))
        perm = meta["perms"][k]
        valid = perm >= 0
        out[perm[valid]] = stream[valid]
    return out


def kernel(**inputs):
    in_maps, meta = _prep(inputs)
    nc = _build(meta)
    res = run_bass_kernel_spmd(nc, in_maps, list(range(NCORES)))
    return _unshard(res.results, meta)



# revision 12
# speedup vs baseline: 1.0180x; 1.0180x over previous
"""Trainium2 Bass kernel for nn_A3TGCN2_EdgeClassifier (GNN message passing).

Math (validated vs reference in fp32): with H0 = 0 the GRU collapses
(R drops out; softmax over one period == 1):
    deg[d] = 1 + sum_{e: dst=d} ew[e];   dinv = deg^-1/2
    Y  = X @ [Wz@lzW[:64] | Wh@lhW[:64]]              (N,128)
    Ys = dinv * Y
    Yagg[d] = dinv[d] * ( sum_e ew[e]*Ys[src[e]] + Ys[d] )
    Z = sigmoid(Yagg[:,:64] + bz');  Ht = tanh(Yagg[:,64:] + bh')
    h = (1-Z)*Ht;  A = h@W1[:64];  B = h@W1[64:] + b1
    out[e] = relu(A[esrc]+B[edst]) @ W2 + b2          (E,2)

Distribution: nodes sharded across 8 cores (12544 each); each core builds
its slice of the (N,128) node tables, all-gathered via collectives. Per-edge
row gathers use the gpsimd dma_gather ucode op (InstDMAGatherAnt), 2048
rows of 256B per instruction (994ns + 0.34ns/row descriptor-gen). dma_gather
takes int16 indices, so edge chunks are bucketed by table quarter (25088
rows) and gathered from a quarter-offset source AP. The GCN aggregation is
sharded by dst ownership: chunks of 128 dst-sorted edges scatter via
one-hot matmul into per-dst-tile PSUM accumulators. The one-hot scatter
matrices (with edge weights folded in) are precomputed on host and DMA-
streamed in bf16, keeping the DVE off the critical path. The edge MLP is
sharded by edge-dst owner: B[edst] is expanded by a host-streamed one-hot
matmul; A[esrc] rows come from batched dma_gather of full AB rows; final
2-col W2 matmuls are paired (block-diagonal W2) to halve matmul count.
"""

import sys

try:
    import concourse.bass as bass  # noqa: F401
except Exception:  # pragma: no cover
    sys.path.insert(0, "/opt/trn_rl_repo")

import numpy as np
import ml_dtypes

import concourse.bass as bass
import concourse.mybir as mybir
from concourse import bacc, tile
from concourse.bass_utils import run_bass_kernel_spmd

BF16 = ml_dtypes.bfloat16
F32 = np.float32

NCORES = 8
N = 100_000
E = 1_600_000
FIN = 80
NLOC = 12544               # padded nodes per core
NPAD = NLOC * NCORES       # 100352
TPC = NLOC // 128          # 98 node tiles per core
GT = 6                     # node tiles per aggregation group (PSUM residency)
QS = NPAD // 4             # table quarter for int16 gather indices (25088)
NG = 16                    # max chunks (x128 rows) per dma_gather call
EPC = E // NCORES          # 200000

dt = mybir.dt


def _mk_calls(stream_q):
    """Split a chunk stream [(chunk_meta, key), ...] into dma_gather calls of
    <=NG chunks with a constant split key (whose last element is the table
    quarter q). Returns list of (q, [chunk_meta,...])."""
    calls = []
    cur_k, cur = None, []
    for meta, key in stream_q:
        if key != cur_k or len(cur) == NG:
            if cur:
                calls.append((cur_k[-1], cur))
            cur_k, cur = key, []
        cur.append(meta)
    if cur:
        calls.append((cur_k[-1], cur))
    return calls


def _slots(t_k, q_k, colof):
    """Vectorized slot assignment: edges -> (col, par) within (t, q) chunks."""
    n = len(t_k)
    key = t_k * 4 + q_k
    order2 = np.lexsort((np.arange(n), key))
    cnt1d = np.bincount(key, minlength=TPC * 4)
    bstart = np.zeros(TPC * 4, np.int64)
    bstart[1:] = np.cumsum(cnt1d)[:-1]
    skey = key[order2]
    pos = np.arange(n) - bstart[skey]
    col = np.zeros(n, np.int64)
    par = np.zeros(n, np.int64)
    col[order2] = colof.reshape(-1)[skey] + pos // 128
    par[order2] = pos % 128
    return col, par


def _prep(inputs):
    """Host-side sharding / graph preprocessing. Returns (in_maps, meta)."""
    x = np.asarray(inputs["x"], F32)[0, :, :, 0]
    ei = np.asarray(inputs["edge_index"]).astype(np.int64)
    src, dst = ei[0], ei[1]
    ew = np.asarray(inputs["edge_weight"], F32)
    esrc = np.asarray(inputs["edge_src"]).astype(np.int64)
    edst = np.asarray(inputs["edge_dst"]).astype(np.int64)

    lzW = np.asarray(inputs["lz_W"], F32)[:64]
    lhW = np.asarray(inputs["lh_W"], F32)[:64]
    Wzp = np.asarray(inputs["Wz"], F32) @ lzW
    Whp = np.asarray(inputs["Wh"], F32) @ lhW
    bzp = np.asarray(inputs["bz"], F32) @ lzW + np.asarray(inputs["lz_b"], F32)
    bhp = np.asarray(inputs["bh"], F32) @ lhW + np.asarray(inputs["lh_b"], F32)
    W1 = np.asarray(inputs["mlp_W1"], F32)
    b1 = np.asarray(inputs["mlp_b1"], F32)
    W2 = np.asarray(inputs["mlp_W2"], F32)
    b2 = np.asarray(inputs["mlp_b2"], F32)

    wfused = np.ascontiguousarray(np.concatenate([Wzp, Whp], 1)).astype(BF16)
    gbias = np.tile(np.concatenate([bzp, bhp])[None, :], (128, 1)).astype(F32)
    w1cat = np.ascontiguousarray(np.concatenate([W1[:64], W1[64:]], 1)).astype(BF16)
    b1row = np.concatenate([np.zeros(64, F32), b1])[None, :].astype(BF16)
    # block-diagonal W2 for paired pO matmuls: rows 0:64 -> cols 0:2,
    # rows 64:128 -> cols 2:4
    w2stk = np.zeros((128, 4), F32)
    w2stk[:64, 0:2] = W2
    w2stk[64:, 2:4] = W2
    w2stk = w2stk.astype(BF16)
    b2rep = np.tile(b2[None, :], (128, 64)).astype(F32)
    ident = np.eye(128, dtype=F32).astype(BF16)
    ones1 = np.ones((1, 128), BF16)

    xpad = np.zeros((NPAD, FIN), F32)
    xpad[:N] = x
    xts = [np.ascontiguousarray(xpad[k * NLOC:(k + 1) * NLOC].T).astype(BF16)
           for k in range(NCORES)]

    counts = np.bincount(dst, minlength=NPAD)
    L = int(counts.max())
    order = np.argsort(dst, kind="stable")
    dsts = dst[order]
    ews = ew[order]
    srcs = src[order]
    cum = np.zeros(NPAD + 1, np.int64)
    cum[1:] = np.cumsum(counts)
    ewpad = np.zeros((NPAD, L), F32)
    posn = np.arange(E) - cum[dsts]
    ewpad[dsts, posn] = ews
    ewpads = [np.ascontiguousarray(
        ewpad[k * NLOC:(k + 1) * NLOC]
        .reshape(TPC, 128, L).transpose(1, 0, 2).reshape(128, TPC * L))
        for k in range(NCORES)]

    # ---- aggregation streams: chunk per (dst tile, src quarter) ----
    k_of = dsts // NLOC
    t_of = (dsts % NLOC) // 128
    q_of = srcs // QS
    cnt = np.zeros((NCORES, TPC, 4), np.int64)
    np.add.at(cnt, (k_of, t_of, q_of), 1)
    capsq = (cnt.max(0) + 127) // 128              # [TPC, 4]
    for t in range(TPC):
        if capsq[t].sum() == 0:
            capsq[t][0] = 1

    ngroups = (TPC + GT - 1) // GT
    groups = [list(range(g * GT, min((g + 1) * GT, TPC))) for g in range(ngroups)]

    # chunk stream order: group g -> quarter q -> tile t in g -> chunk cc
    # chunk meta: (c, t, q) with start/stop flags computed from per-tile order
    stream = []                 # (c, t, q)
    c = 0
    for grp in groups:
        for q in range(4):
            for t in grp:
                for _ in range(int(capsq[t][q])):
                    stream.append((c, t, q))
                    c += 1
    CTOT = c
    colof = np.full((TPC, 4), -1, np.int64)
    for (cc, t, q) in stream:
        if colof[t][q] < 0:
            colof[t][q] = cc

    # start/stop flags per chunk (psum accumulate lifetime = per tile)
    first_of_t, last_of_t = {}, {}
    for (cc, t, q) in stream:
        if t not in first_of_t:
            first_of_t[t] = cc
        last_of_t[t] = cc
    chunk_flags = [(cc, t, q, cc == first_of_t[t], cc == last_of_t[t])
                   for (cc, t, q) in stream]

    # gather calls (same for all cores); split on (psum group, quarter)
    gidx_of_tile = {}
    for gi, grp in enumerate(groups):
        for t in grp:
            gidx_of_tile[t] = gi
    calls_agg = _mk_calls(
        [((cc, t, cc == first_of_t[t], cc == last_of_t[t]), (gidx_of_tile[t], q))
         for (cc, t, q) in stream])
    AIW = sum(len(ch) * 8 for _, ch in calls_agg)   # int16 cols in idx stream

    # per-core stream data
    agg = []
    for k in range(NCORES):
        sel = slice(cum[k * NLOC], cum[(k + 1) * NLOC])
        d_k = dsts[sel]
        s_k = srcs[sel]
        e_k = ews[sel]
        t_k = (d_k - k * NLOC) // 128
        q_k = s_k // QS
        slot_col, slot_par = _slots(t_k, q_k, colof)
        srcg = np.zeros((128, CTOT), np.int64)   # global src per slot (pad=q*QS)
        # init pads to the chunk's quarter base so rel idx is 0
        for (cc, t, q) in stream:
            srcg[:, cc] = q * QS
        srcg[slot_par, slot_col] = s_k
        # host-built one-hot scatter stream: ohagg[p=slot, cc, f=dstrel] = ew
        # (pads have ew=0 so their f=0 writes are harmless zeros)
        ohagg = np.zeros((128, CTOT, 128), BF16)
        ohagg[slot_par, slot_col, ((d_k - k * NLOC) % 128)] = e_k.astype(BF16)
        # idx16 stream in call order, wrapped [16, w*8] replicated to 128 rows
        blocks = []
        for q, chs in calls_agg:
            cols = [m[0] for m in chs]
            vals = (srcg[:, cols].T.reshape(-1) - q * QS).astype(np.int16)
            w16 = vals.reshape(-1, 16).T            # [16, w*8]
            blocks.append(np.tile(w16, (8, 1)))
        aggidx = np.concatenate(blocks, axis=1)
        assert aggidx.shape == (128, AIW)
        agg.append((ohagg.reshape(128, CTOT * 128), aggidx))

    # ---- MLP streams: chunk per (edst tile, esrc quarter), B expanded ----
    morder = np.argsort(edst, kind="stable")
    medst = edst[morder]
    mesrc = esrc[morder]
    mcum = np.zeros(NPAD + 1, np.int64)
    mcum[1:] = np.cumsum(np.bincount(medst, minlength=NPAD))
    mk_of = medst // NLOC
    mt_of = (medst % NLOC) // 128
    mq_of = mesrc // QS
    cntm = np.zeros((NCORES, TPC, 4), np.int64)
    np.add.at(cntm, (mk_of, mt_of, mq_of), 1)
    capm = (cntm.max(0) + 127) // 128               # [TPC, 4], 0 allowed

    # chunk order: quarter-major, then tile
    mstream = []
    mcolof = np.full((TPC, 4), -1, np.int64)
    c = 0
    for q in range(4):
        for t in range(TPC):
            for _ in range(int(capm[t][q])):
                if mcolof[t][q] < 0:
                    mcolof[t][q] = c
                mstream.append((c, t, q))
                c += 1
    CTOTM = c
    calls_mlp = _mk_calls([((cc, t), (q,)) for (cc, t, q) in mstream])
    MIW = sum(len(ch) * 8 for _, ch in calls_mlp)

    mlp = []
    for k in range(NCORES):
        sel = slice(mcum[k * NLOC], mcum[(k + 1) * NLOC])
        d_k = medst[sel]
        s_k = mesrc[sel]
        t_k = (d_k - k * NLOC) // 128
        q_k = s_k // QS
        slot_col, slot_par = _slots(t_k, q_k, mcolof)
        srcg = np.zeros((128, CTOTM), np.int64)
        for (cc, t, q) in mstream:
            srcg[:, cc] = q * QS
        perm = np.full(CTOTM * 128, -1, np.int64)
        srcg[slot_par, slot_col] = s_k
        perm[slot_col * 128 + slot_par] = morder[sel]
        # host-built B-expansion one-hot: ohmlp[p=dstrel, cg, f=slot] = 1
        # (pads land on p=0: adds B[row0 of tile] to a slot nobody reads)
        ohmlp = np.zeros((128, CTOTM, 128), BF16)
        ohmlp[((d_k - k * NLOC) % 128), slot_col, slot_par] = np.float32(1.0)
        blocks = []
        for q, chs in calls_mlp:
            cols = [m[0] for m in chs]
            vals = (srcg[:, cols].T.reshape(-1) - q * QS).astype(np.int16)
            w16 = vals.reshape(-1, 16).T
            blocks.append(np.tile(w16, (8, 1)))
        mlpidx = np.concatenate(blocks, axis=1)
        assert mlpidx.shape == (128, MIW)
        mlp.append((ohmlp.reshape(128, CTOTM * 128), mlpidx, perm))

    NPIECE = (CTOTM + 63) // 64

    consts = dict(wfused=wfused, gbias=gbias, w1cat=w1cat, b1row=b1row,
                  w2stk=w2stk, b2rep=b2rep, ident=ident, ones1=ones1)
    in_maps = []
    for k in range(NCORES):
        ohagg, aggidx = agg[k]
        ohmlp, mlpidx, _ = mlp[k]
        in_maps.append(dict(
            xt=xts[k], ewpad=ewpads[k], ohagg=ohagg,
            aggidx=aggidx, ohmlp=ohmlp, mlpidx=mlpidx, **consts))

    meta = dict(L=L, groups=groups, CTOT=CTOT, CTOTM=CTOTM,
                calls_agg=calls_agg, calls_mlp=calls_mlp, AIW=AIW, MIW=MIW,
                NPIECE=NPIECE, perms=[m[2] for m in mlp])
    return in_maps, meta


def _build(meta):
    L = meta["L"]
    groups = meta["groups"]
    CTOT = meta["CTOT"]
    CTOTM = meta["CTOTM"]
    calls_agg = meta["calls_agg"]
    calls_mlp = meta["calls_mlp"]
    AIW = meta["AIW"]
    MIW = meta["MIW"]
    NPIECE = meta["NPIECE"]

    nc = bacc.Bacc("TRN2", target_bir_lowering=False, debug=False,
                   num_devices=NCORES, num_swdge_queues=4)

    xt_d = nc.dram_tensor("xt", [FIN, NLOC], dt.bfloat16, kind="ExternalInput")
    ewpad_d = nc.dram_tensor("ewpad", [128, TPC * L], dt.float32, kind="ExternalInput")
    ohagg_d = nc.dram_tensor("ohagg", [128, CTOT * 128], dt.bfloat16, kind="ExternalInput")
    aggidx_d = nc.dram_tensor("aggidx", [128, AIW], dt.int16, kind="ExternalInput")
    ohmlp_d = nc.dram_tensor("ohmlp", [128, CTOTM * 128], dt.bfloat16, kind="ExternalInput")
    mlpidx_d = nc.dram_tensor("mlpidx", [128, MIW], dt.int16, kind="ExternalInput")
    wfused_d = nc.dram_tensor("wfused", [FIN, 128], dt.bfloat16, kind="ExternalInput")
    gbias_d = nc.dram_tensor("gbias", [128, 128], dt.float32, kind="ExternalInput")
    w1cat_d = nc.dram_tensor("w1cat", [64, 128], dt.bfloat16, kind="ExternalInput")
    b1row_d = nc.dram_tensor("b1row", [1, 128], dt.bfloat16, kind="ExternalInput")
    w2stk_d = nc.dram_tensor("w2stk", [128, 4], dt.bfloat16, kind="ExternalInput")
    b2rep_d = nc.dram_tensor("b2rep", [128, 128], dt.float32, kind="ExternalInput")
    ident_d = nc.dram_tensor("ident", [128, 128], dt.bfloat16, kind="ExternalInput")
    ones1_d = nc.dram_tensor("ones1", [1, 128], dt.bfloat16, kind="ExternalInput")
    out_d = nc.dram_tensor("out", [NPIECE * 128, 128], dt.float32, kind="ExternalOutput")

    ysloc = nc.dram_tensor("ysloc", [NLOC, 128], dt.bfloat16)
    ysfull = nc.dram_tensor("ysfull", [NPAD, 128], dt.bfloat16, addr_space="Shared")
    abloc = nc.dram_tensor("abloc", [NLOC, 128], dt.bfloat16)
    abfull = nc.dram_tensor("abfull", [NPAD, 128], dt.bfloat16, addr_space="Shared")

    rg = [list(range(NCORES))]

    with tile.TileContext(nc) as tc:
        with (
            tc.tile_pool(name="const", bufs=1) as cp,
            tc.tile_pool(name="persist", bufs=1) as pp,
        ):
            wfused_sb = cp.tile([FIN, 128], dt.bfloat16)
            gbias_sb = cp.tile([128, 128], dt.float32)
            w1cat_sb = cp.tile([64, 128], dt.bfloat16)
            b1row_sb = cp.tile([1, 128], dt.bfloat16)
            w2stk_sb = cp.tile([128, 4], dt.bfloat16)
            b2rep_sb = cp.tile([128, 128], dt.float32)
            ident_sb = cp.tile([128, 128], dt.bfloat16)
            ones1_sb = cp.tile([1, 128], dt.bfloat16)
            for sb, d in [
                (wfused_sb, wfused_d), (gbias_sb, gbias_d), (w1cat_sb, w1cat_d),
                (b1row_sb, b1row_d), (w2stk_sb, w2stk_d), (b2rep_sb, b2rep_d),
                (ident_sb, ident_d), (ones1_sb, ones1_d),
            ]:
                nc.sync.dma_start(out=sb[:], in_=d[:])

            dinv = pp.tile([128, TPC], dt.float32)
            dinv2 = pp.tile([128, TPC], dt.float32)
            ys2b = pp.tile([128, TPC * 128], dt.bfloat16)

            # ---------------- deg / dinv ----------------
            with tc.tile_pool(name="degp", bufs=1) as dp:
                ewpad_sb = dp.tile([128, TPC * L], dt.float32)
                nc.sync.dma_start(out=ewpad_sb[:], in_=ewpad_d[:])
                deg = dp.tile([128, TPC], dt.float32)
                nc.vector.tensor_reduce(
                    deg[:], ewpad_sb[:].rearrange("p (t l) -> p t l", t=TPC),
                    axis=mybir.AxisListType.X, op=mybir.AluOpType.add)
                sq = dp.tile([128, TPC], dt.float32)
                nc.scalar.activation(sq[:], deg[:],
                                     mybir.ActivationFunctionType.Sqrt, bias=1.0)
                nc.vector.reciprocal(dinv[:], sq[:])
                nc.vector.tensor_mul(dinv2[:], dinv[:], dinv[:])

            # ---------------- node phase ----------------
            with (
                tc.tile_pool(name="xtp", bufs=1) as xtp,
                tc.tile_pool(name="ysp", bufs=1) as ysp,
                tc.tile_pool(name="pY", bufs=4, space="PSUM") as pYp,
            ):
                xt_all = xtp.tile([FIN, NLOC], dt.bfloat16)
                nc.sync.dma_start(out=xt_all[:], in_=xt_d[:])
                ys_all = ysp.tile([128, TPC * 128], dt.bfloat16)
                for t in range(TPC):
                    pY = pYp.tile([128, 128], dt.float32)
                    nc.tensor.matmul(pY[:], xt_all[:, t * 128:(t + 1) * 128],
                                     wfused_sb[:], start=True, stop=True)
                    nc.vector.tensor_scalar_mul(
                        ys_all[:, t * 128:(t + 1) * 128], pY[:], dinv[:, t:t + 1])
                    nc.vector.scalar_tensor_tensor(
                        ys2b[:, t * 128:(t + 1) * 128], pY[:], dinv2[:, t:t + 1],
                        gbias_sb[:], op0=mybir.AluOpType.mult, op1=mybir.AluOpType.add)
                nc.sync.dma_start(
                    out=ysloc[:].rearrange("(t p) f -> p t f", p=128),
                    in_=ys_all[:].rearrange("p (t f) -> p t f", t=TPC))

            nc.gpsimd.collective_compute(
                "AllGather", mybir.AluOpType.bypass, replica_groups=rg,
                ins=[ysloc[:]], outs=[ysfull[:]])

            # ---------------- aggregation + gates + A|B ----------------
            with (
                tc.tile_pool(name="aggstream", bufs=1) as asp,
                tc.tile_pool(name="gat", bufs=6) as gatp,
                tc.tile_pool(name="oh", bufs=6) as ohp,
                tc.tile_pool(name="gate", bufs=3) as gp_,
                tc.tile_pool(name="abp", bufs=3) as abp,
                tc.tile_pool(name="pAgg", bufs=GT, space="PSUM") as pAgg,
                tc.tile_pool(name="pT", bufs=1, space="PSUM") as pT,
                tc.tile_pool(name="pAB", bufs=1, space="PSUM") as pAB,
            ):
                aggidx_sb = asp.tile([128, AIW], dt.int16)
                nc.sync.dma_start(out=aggidx_sb[:], in_=aggidx_d[:])

                # group calls by psum group: tiles of group g span calls whose
                # chunks' tiles are all in g (stream order guarantees this)
                gidx_of_tile = {}
                for gi, grp in enumerate(groups):
                    for t in grp:
                        gidx_of_tile[t] = gi
                iw = 0
                cur_g = -1
                ps = {}
                for ci, (q, chs) in enumerate(calls_agg):
                    w = len(chs)
                    gi = gidx_of_tile[chs[0][1]]
                    if gi != cur_g:
                        # allocate psum tiles for the new group
                        cur_g = gi
                        ps = {t: pAgg.tile([128, 128], dt.float32, tag="aggpsum",
                                           name=f"ps_g{gi}_t{t}")
                              for t in groups[gi]}
                    gt_ = gatp.tile([128, NG * 128], dt.bfloat16, tag="gat")
                    nc.gpsimd.dma_gather(
                        out_ap=gt_[:, :w * 128].rearrange("p (b f) -> p b f", b=w),
                        in_ap=ysfull[q * QS:(q + 1) * QS, :],
                        idxs_ap=aggidx_sb[:, iw:iw + w * 8],
                        num_idxs=w * 128, num_idxs_reg=w * 128,
                        elem_size=128, transpose=False,
                        single_packet=False, queue_num=ci % 4)
                    iw += w * 8
                    cc0 = chs[0][0]
                    oh = ohp.tile([128, NG * 128], dt.bfloat16, tag="oh")
                    nc.sync.dma_start(
                        out=oh[:, :w * 128],
                        in_=ohagg_d[:, cc0 * 128:(cc0 + w) * 128])
                    for j, (cc, t, st, sp) in enumerate(chs):
                        nc.tensor.matmul(ps[t][:], oh[:, j * 128:(j + 1) * 128],
                                         gt_[:, j * 128:(j + 1) * 128],
                                         start=st, stop=sp)
                        if sp:
                            # epilogue for tile t
                            yagg = gp_.tile([128, 128], dt.float32, tag="yagg")
                            nc.vector.scalar_tensor_tensor(
                                yagg[:], ps[t][:], dinv[:, t:t + 1],
                                ys2b[:, t * 128:(t + 1) * 128],
                                op0=mybir.AluOpType.mult, op1=mybir.AluOpType.add)
                            zt = gp_.tile([128, 64], dt.float32, tag="zt")
                            ht = gp_.tile([128, 64], dt.float32, tag="ht")
                            nc.scalar.activation(zt[:], yagg[:, 0:64],
                                                 mybir.ActivationFunctionType.Sigmoid)
                            nc.scalar.activation(ht[:], yagg[:, 64:128],
                                                 mybir.ActivationFunctionType.Tanh)
                            zh = gp_.tile([128, 64], dt.float32, tag="zh")
                            nc.vector.tensor_mul(zh[:], zt[:], ht[:])
                            hbf = gp_.tile([128, 64], dt.bfloat16, tag="hbf")
                            nc.vector.tensor_sub(hbf[:], ht[:], zh[:])
                            psT = pT.tile([64, 128], dt.bfloat16)
                            nc.tensor.transpose(psT[:], hbf[:], ident_sb[:])
                            hT = gp_.tile([64, 128], dt.bfloat16, tag="hT")
                            nc.vector.tensor_copy(hT[:], psT[:])
                            psAB = pAB.tile([128, 128], dt.float32)
                            nc.tensor.matmul(psAB[:], ones1_sb[:], b1row_sb[:],
                                             start=True, stop=False)
                            nc.tensor.matmul(psAB[:], hT[:], w1cat_sb[:],
                                             start=False, stop=True)
                            ab = abp.tile([128, 128], dt.bfloat16)
                            nc.scalar.copy(ab[:], psAB[:])
                            nc.sync.dma_start(
                                out=abloc[t * 128:(t + 1) * 128, :], in_=ab[:])

            nc.gpsimd.collective_compute(
                "AllGather", mybir.AluOpType.bypass, replica_groups=rg,
                ins=[abloc[:]], outs=[abfull[:]])

            # ---------------- MLP phase (A gathered, B expanded) ----------------
            with (
                tc.tile_pool(name="mstream", bufs=1) as msp,
                tc.tile_pool(name="bwinp", bufs=1) as bwp,
                tc.tile_pool(name="ag", bufs=6) as agp,
                tc.tile_pool(name="ohn", bufs=6) as ohnp,
                tc.tile_pool(name="hT2", bufs=8) as hT2p,
                tc.tile_pool(name="op", bufs=3) as op_,
                tc.tile_pool(name="pO", bufs=2, space="PSUM") as pOp,
                tc.tile_pool(name="pE", bufs=4, space="PSUM") as pEp,
            ):
                bwin_all = bwp.tile([128, TPC * 64], dt.bfloat16)
                nc.sync.dma_start(
                    out=bwin_all[:].rearrange("p (t f) -> p t f", t=TPC),
                    in_=abloc[:, 64:128].rearrange("(t p) f -> p t f", p=128))
                mlpidx_sb = msp.tile([128, MIW], dt.int16)
                nc.sync.dma_start(out=mlpidx_sb[:], in_=mlpidx_d[:])
                MG = 64
                pO = None
                hT2 = None
                iw = 0
                for ci, (q, chs) in enumerate(calls_mlp):
                    w = len(chs)
                    ag = agp.tile([128, NG * 128], dt.bfloat16, tag="ag")
                    nc.gpsimd.dma_gather(
                        out_ap=ag[:, :w * 128].rearrange("p (b f) -> p b f", b=w),
                        in_ap=abfull[q * QS:(q + 1) * QS, :],
                        idxs_ap=mlpidx_sb[:, iw:iw + w * 8],
                        num_idxs=w * 128, num_idxs_reg=w * 128,
                        elem_size=128, transpose=False,
                        single_packet=False, queue_num=ci % 4)
                    iw += w * 8
                    cg0 = chs[0][0]
                    ohn = ohnp.tile([128, NG * 128], dt.bfloat16, tag="ohn")
                    nc.sync.dma_start(
                        out=ohn[:, :w * 128],
                        in_=ohmlp_d[:, cg0 * 128:(cg0 + w) * 128])
                    for j, (cg, t) in enumerate(chs):
                        b = cg % MG
                        if b == 0:
                            pO = pOp.tile([128, 128], dt.float32)
                        psE = pEp.tile([64, 128], dt.float32)
                        nc.tensor.matmul(psE[:], bwin_all[:, t * 64:(t + 1) * 64],
                                         ohn[:, j * 128:(j + 1) * 128],
                                         start=True, stop=False)
                        nc.tensor.matmul(psE[:], ag[:, j * 128:j * 128 + 64],
                                         ident_sb[:], start=False, stop=True)
                        half = cg % 2
                        if half == 0:
                            hT2 = hT2p.tile([128, 128], dt.bfloat16, tag="hT2")
                        nc.scalar.activation(hT2[64 * half:64 * half + 64, :],
                                             psE[:],
                                             mybir.ActivationFunctionType.Relu)
                        if half == 1:
                            nc.tensor.matmul(pO[:, 2 * b - 2:2 * b + 2], hT2[:],
                                             w2stk_sb[:], start=True, stop=True)
                        elif cg == CTOTM - 1:
                            # unpaired tail: zero the odd half, then matmul
                            nc.vector.memset(hT2[64:128, :], 0.0)
                            nc.tensor.matmul(pO[:, 2 * b:2 * b + 4], hT2[:],
                                             w2stk_sb[:], start=True, stop=True)
                        if b == MG - 1 or cg == CTOTM - 1:
                            nb = b + 1
                            osb = op_.tile([128, 128], dt.float32)
                            nc.vector.tensor_add(osb[:, :2 * nb], pO[:, :2 * nb],
                                                 b2rep_sb[:, :2 * nb])
                            qq = cg // MG
                            nc.sync.dma_start(
                                out=out_d[qq * 128:(qq + 1) * 128, :], in_=osb[:])

    nc.compile()
    return nc


def _unshard(results, meta):
    CTOTM = meta["CTOTM"]
    out = np.zeros((E, 2), F32)
    for k in range(NCORES):
        outd = np.asarray(results[k]["out"])
        nslots = CTOTM * 128
        stream = np.empty((nslots, 2), F32)
        for q in range((CTOTM + 63) // 64):
            kk = min(64, CTOTM - q * 64)
            blk = outd[q * 128:(q + 1) * 128, :2 * kk]
            stream[q * 64 * 128:(q * 64 + kk) * 128] = (
                blk.reshape(128, kk, 2).transpose(1, 0, 2).reshape(kk * 128, 2))
        perm = meta["perms"][k]
        valid = perm >= 0
        out[perm[valid]] = stream[valid]
    return out


def kernel(**inputs):
    in_maps, meta = _prep(inputs)
    nc = _build(meta)
    res = run_bass_kernel_spmd(nc, in_maps, list(range(NCORES)))
    return _unshard(res.results, meta)

